# revision 48
# baseline (speedup 1.0000x reference)
"""GCN mean-aggregation (DGL copy_src -> mean by dst) on 8 NeuronCores.

Strategy (dst-sharded, no collectives):
  - Host: edges are assigned to the core owning their dst row (core c owns
    rows [c*12500, (c+1)*12500)).  Within a core, dst nodes form 98 buckets
    of 128; src rows are split into 4 groups of 25000 so gather indices fit
    int16 (dma_gather requirement).  Edges are sorted by
    (bucket-wave, src-group, bucket, src) and each (bucket, group) run is
    padded to a static number of 128-edge tiles (max over the 8 cores), so a
    single program serves all cores.  Pad edges gather a garbage row and are
    masked out by a zero one-hot row (dst_local = 128).
  - Embeddings are pre-cast to bf16 and padded to 256B rows ([N, 128] bf16,
    features in cols 0:64): dma_gather requires a 256B-aligned source stride.
    Only the 64 real features (128B) are gathered per edge (elem_size=64,
    elem_step=128; the stock 256B elem_size assert is relaxed at import).
  - Counts are folded host-side into a per-core reciprocal table rec[p, b] =
    1/max(indeg, 1); no count matmul on device.
  - Device (identical program per core):
      * per (wave of 8 buckets) x (src group): batched dma_gather of bf16
        embedding rows (128B each) into SBUF, rotating 4 SWDGE queues
      * per wave: ONE DVE tensor_tensor(is_equal) with stride-0 broadcast
        APs builds the bf16 one-hots for every tile in the wave
      * per edge-tile: psum[:, :64] += onehot^T @ msgs   (bf16 matmul)
      * per bucket: Activation-engine copy psum*rec -> SBUF; DMA out
  - Host: concatenate the 8 per-core [12500, 64] outputs.
"""

import sys
from contextlib import ExitStack

import numpy as np
import ml_dtypes

sys.path.insert(0, "/opt/trn_rl_repo")

import concourse.bass as bass  # noqa: E402
import concourse.mybir as mybir  # noqa: E402
import concourse.tile as tile  # noqa: E402
from concourse import bacc  # noqa: E402
from concourse.bass_utils import run_bass_kernel_spmd  # noqa: E402

# dma_gather asserts elem_size_bytes % 256 == 0, but the ISA only needs the
# source stride (elem_step) 256B-aligned; relax to 128B so we can gather just
# the 64 real bf16 features (128B) from each 256B-strided row.
import inspect as _inspect  # noqa: E402
import textwrap as _textwrap  # noqa: E402

_src = _textwrap.dedent(_inspect.getsource(bass.BassGpSimd.dma_gather))
_src = _src.replace("elem_size_bytes % 256 == 0", "elem_size_bytes % 128 == 0")
_ns = dict(bass.__dict__)
exec(_src, _ns)  # noqa: S102
bass.BassGpSimd.dma_gather = _ns["dma_gather"]

N_NODES = 100000
N_EDGES = 1000000
D_FEAT = 64
N_CORES = 8
NODES_PER_CORE = N_NODES // N_CORES  # 12500
BUCKET = 128  # dst nodes per psum bucket (= one-hot free dim)
N_GROUPS = 4  # src-row groups (int16 index range for dma_gather)
WAVE = 8  # buckets per gather wave
ROW = 128  # padded bf16 row length (256B dma_gather granularity)

BF16 = ml_dtypes.bfloat16


def _schedule(cnt_max, npc, bucket, wave):
    """Static schedule from per-(bucket, group) max edge counts.

    cnt_max: [nb, ngroups] max edge count over cores.
    Returns dict with tiles-per-region, waves, per-call and per-bucket info.
    """
    nb, ngroups = cnt_max.shape
    tbg = -(-cnt_max // 128)  # [nb, ngroups]
    for b in range(nb):
        if tbg[b].sum() == 0:
            tbg[b, 0] = 1  # ensure psum gets reset even for empty buckets

    waves = [range(w, min(w + wave, nb)) for w in range(0, nb, wave)]
    # region order: (wave, group, bucket-in-wave)
    region_tile0 = np.zeros((nb, ngroups), np.int64)
    calls = []  # [wave][group] -> (tile0, ntiles)
    t = 0
    for wv in waves:
        wcalls = []
        for g in range(ngroups):
            c0 = t
            for b in wv:
                region_tile0[b, g] = t
                t += int(tbg[b, g])
            wcalls.append((c0, t - c0))
        calls.append(wcalls)
    nt = t
    return {
        "tbg": tbg,
        "waves": waves,
        "region_tile0": region_tile0,
        "calls": calls,
        "nt": nt,
    }


def _prep(src, dst, n_nodes, n_cores, npc, bucket, ngroups, wave):
    """Sort/group/pad edges; build per-core device inputs + static schedule."""
    src = np.asarray(src, dtype=np.int64)
    dst = np.asarray(dst, dtype=np.int64)
    gsz = n_nodes // ngroups
    nb = -(-npc // bucket)
    nw = -(-nb // wave)

    core = dst // npc
    b = (dst - core * npc) // bucket
    g = src // gsz
    w = b // wave

    cnt = np.zeros((n_cores, nb, ngroups), np.int64)
    np.add.at(cnt, (core, b, g), 1)
    sched = _schedule(cnt.max(axis=0), npc, bucket, wave)
    tbg, region_tile0, nt = sched["tbg"], sched["region_tile0"], sched["nt"]
    nslot = nt * 128

    # global sort by (core, wave, group, bucket, src)
    key = (((core * nw + w) * ngroups + g) * nb + b)
    order = np.lexsort((src, key))
    ss, ks = src[order], key[order]
    dl = (dst - (core * npc + b * bucket))[order]  # dst_local in [0, bucket)

    kcnt = np.bincount(ks, minlength=n_cores * nw * ngroups * nb)
    kstart = np.zeros(kcnt.shape[0] + 1, np.int64)
    np.cumsum(kcnt, out=kstart[1:])
    rank = np.arange(ss.shape[0], dtype=np.int64) - kstart[ks]

    slot_base = region_tile0 * 128  # [nb, ngroups], within-core slot offset
    bo, go, co = b[order], g[order], core[order]
    pos = co * nslot + slot_base[bo, go] + rank

    # per-slot group id (for pad values), same for every core
    slot_group = np.zeros(nslot, np.int64)
    for bb in range(nb):
        for gg in range(ngroups):
            t0 = region_tile0[bb, gg] * 128
            slot_group[t0 : t0 + tbg[bb, gg] * 128] = gg

    src_slot = np.tile((slot_group + 1) * gsz - 1, n_cores)  # pad: last row of group
    dstloc = np.full(n_cores * nslot, float(bucket), np.float32)
    src_slot[pos] = ss
    dstloc[pos] = dl.astype(np.float32)

    idx16 = (src_slot - np.tile(slot_group * gsz, n_cores)).astype(np.int16)
    # wrapped index layout: idx j -> partition j%16, col j//16 (x8 replicas)
    idx16 = idx16.reshape(n_cores, nt * 8, 16)
    idxtab = np.ascontiguousarray(idx16.transpose(0, 2, 1))  # [C, 16, nt*8]
    idxtab = np.tile(idxtab, (1, 8, 1))  # [C, 128, nt*8]

    dst_t = np.ascontiguousarray(
        dstloc.reshape(n_cores, nt, 128).transpose(0, 2, 1)
    ).astype(BF16)  # [C, 128, nt]

    # per-core reciprocal table rec[p, b] = 1/max(indeg(core*npc + b*128 + p), 1)
    indeg = np.bincount(dst, minlength=n_nodes).astype(np.float32)
    rec = 1.0 / np.maximum(indeg, 1.0)
    nb_rows = nb * 128
    rec_t = np.ones((n_cores, nb_rows), np.float32)
    for c in range(n_cores):
        rec_t[c, :npc] = rec[c * npc : (c + 1) * npc]
    rec_t = np.ascontiguousarray(
        rec_t.reshape(n_cores, nb, 128).transpose(0, 2, 1)
    )  # [C, 128, nb]
    return idxtab, dst_t, rec_t, sched


def _build(n_nodes, d_feat, npc, bucket, ngroups, sched):
    """Build the (per-core) Bass program."""
    gsz = n_nodes // ngroups
    nb = -(-npc // bucket)
    nt = sched["nt"]
    tbg, region_tile0 = sched["tbg"], sched["region_tile0"]
    f32 = mybir.dt.float32
    bf16 = mybir.dt.bfloat16
    i16 = mybir.dt.int16

    # max tiles in any wave (for pool sizing)
    wave_spans = []
    for wvi, wv in enumerate(sched["waves"]):
        wt0 = sched["calls"][wvi][0][0]
        wt = sum(c[1] for c in sched["calls"][wvi])
        wave_spans.append((wt0, wt))

    nc = bacc.Bacc(
        "TRN2", target_bir_lowering=False, debug=False, num_swdge_queues=4
    )
    emb = nc.dram_tensor("emb", [n_nodes, ROW], bf16, kind="ExternalInput")
    idx_t = nc.dram_tensor("idx_t", [128, nt * 8], i16, kind="ExternalInput")
    dst_t = nc.dram_tensor("dst_t", [128, nt], bf16, kind="ExternalInput")
    rec_t = nc.dram_tensor("rec_t", [128, nb], f32, kind="ExternalInput")
    out = nc.dram_tensor("out", [npc, d_feat], f32, kind="ExternalOutput")

    with tile.TileContext(nc) as tc, ExitStack() as ctx:
        const_p = ctx.enter_context(tc.tile_pool(name="const", bufs=1))
        idx_p = ctx.enter_context(tc.tile_pool(name="idx", bufs=1))
        msgs_p = ctx.enter_context(tc.tile_pool(name="msgs", bufs=3))
        oh_p = ctx.enter_context(tc.tile_pool(name="oh", bufs=3))
        ps_p = ctx.enter_context(tc.tile_pool(name="ps", bufs=8, space="PSUM"))
        outp_p = ctx.enter_context(tc.tile_pool(name="outp", bufs=4))

        # idx/dst tables are loaded per wave so wave 0's gathers start early
        idxall = idx_p.tile([128, nt * 8], i16)
        dstall = idx_p.tile([128, nt], bf16)
        # dummy 128-idx gather warms up the SWDGE ucode library while the
        # idx tables stream in (group 0 so even garbage idx stay in bounds)
        warm = const_p.tile([128, 16], i16)
        nc.gpsimd.memset(warm[:], 0)
        scratch = const_p.tile([128, d_feat], bf16)
        nc.gpsimd.dma_gather(
            out_ap=scratch[:].rearrange("p (t e) -> p t e", e=d_feat),
            in_ap=emb[0:gsz, 0:d_feat],
            idxs_ap=warm[:, 0:8],
            num_idxs=128,
            num_idxs_reg=128,
            elem_size=d_feat,
            elem_step=ROW,
            queue_num=0,
        )
        # two-stage load: wave 0's slice first (its gathers start right away),
        # then the rest in one DMA each (no per-wave HWDGE contention with
        # gather drain at wave boundaries)
        w0t = sum(c[1] for c in sched["calls"][0])
        nc.sync.dma_start(out=idxall[:, : w0t * 8], in_=idx_t[:, : w0t * 8])
        nc.sync.dma_start(out=dstall[:, :w0t], in_=dst_t[:, :w0t])
        nc.sync.dma_start(out=idxall[:, w0t * 8 :], in_=idx_t[:, w0t * 8 :])
        nc.sync.dma_start(out=dstall[:, w0t:], in_=dst_t[:, w0t:])
        recall = idx_p.tile([128, nb], f32)
        nc.sync.dma_start(out=recall[:], in_=rec_t[:, :])
        iota_i = const_p.tile([128, bucket], mybir.dt.int32)
        nc.gpsimd.iota(iota_i[:], pattern=[[1, bucket]], base=0, channel_multiplier=0)
        iota_b = const_p.tile([128, bucket], bf16)
        nc.vector.tensor_copy(out=iota_b[:], in_=iota_i[:])

        qn = 0
        for wvi, wv in enumerate(sched["waves"]):
            wt0, wt = wave_spans[wvi]
            # one batched one-hot build for the whole wave:
            # oh[p, t, f] = (iota[p, f] == dstall[p, wt0 + t])
            oh = oh_p.tile([128, wt * bucket], bf16, tag="oh")
            nc.vector.tensor_tensor(
                out=oh[:].rearrange("p (t f) -> p t f", f=bucket),
                in0=iota_b[:].unsqueeze(1).broadcast_to([128, wt, bucket]),
                in1=dstall[:, wt0 : wt0 + wt]
                .unsqueeze(2)
                .broadcast_to([128, wt, bucket]),
                op=mybir.AluOpType.is_equal,
            )
            msgs = {}
            call0 = {}
            for gg in range(ngroups):
                t0, ntl = sched["calls"][wvi][gg]
                call0[gg] = t0
                if ntl == 0:
                    continue
                m = msgs_p.tile([128, ntl * d_feat], bf16, tag=f"msgs{gg}")
                msgs[gg] = m
                # dma_gather is limited to 1024 indices (8 tiles) per call
                for sc in range(0, ntl, 8):
                    k = min(8, ntl - sc)
                    ts = t0 + sc
                    nc.gpsimd.dma_gather(
                        out_ap=m[
                            :, sc * d_feat : (sc + k) * d_feat
                        ].rearrange("p (t e) -> p t e", e=d_feat),
                        in_ap=emb[gg * gsz : (gg + 1) * gsz, 0:d_feat],
                        idxs_ap=idxall[:, ts * 8 : (ts + k) * 8],
                        num_idxs=k * 128,
                        num_idxs_reg=k * 128,
                        elem_size=d_feat,
                        elem_step=ROW,
                        queue_num=qn // 2,
                    )
                    qn = (qn + 1) % 8  # 2 calls per queue: queue = qn // 2
            # group-major matmul order: buckets' psum chains interleave, so
            # g0..g2 matmuls run while later groups' gathers are in flight
            # (bucket-major would serialize the whole wave behind g3's DMA)
            psums = {}
            for bb in wv:
                psum = ps_p.tile([bucket, d_feat], f32)
                psums[bb] = psum
            first = {bb: True for bb in wv}
            last_pass = {
                bb: max(
                    (gg, int(tbg[bb, gg]) - 1)
                    for gg in range(ngroups)
                    if int(tbg[bb, gg]) > 0
                )
                for bb in wv
            }
            for gg in range(ngroups):
                for bb in wv:
                    for j in range(int(tbg[bb, gg])):
                        t = int(region_tile0[bb, gg]) + j
                        off = int(t - call0[gg]) * d_feat
                        nc.tensor.matmul(
                            out=psums[bb][:],
                            lhsT=oh[:, (t - wt0) * bucket : (t - wt0 + 1) * bucket],
                            rhs=msgs[gg][:, off : off + d_feat],
                            start=first[bb],
                            stop=(gg, j) == last_pass[bb],
                        )
                        first[bb] = False
            # one output tile per wave; a single strided DMA writes all rows
            nw_b = len(wv)
            ow = outp_p.tile([bucket, nw_b * d_feat], f32, tag="ow")
            for bi, bb in enumerate(wv):
                nc.scalar.mul(
                    out=ow[:, bi * d_feat : (bi + 1) * d_feat],
                    in_=psums[bb][:],
                    mul=recall[:, bb : bb + 1],
                )
            r0 = wv[0] * bucket
            nfull = min(nw_b, (npc - r0) // bucket)
            if nfull > 0:
                nc.sync.dma_start(
                    out=out[r0 : r0 + nfull * bucket, :].rearrange(
                        "(b p) f -> p b f", b=nfull
                    ),
                    in_=ow[:, : nfull * d_feat].rearrange(
                        "p (b f) -> p b f", f=d_feat
                    ),
                )
            if nfull < nw_b:  # partial last bucket
                bb = wv[nfull]
                nrows = npc - bb * bucket
                nc.sync.dma_start(
                    out=out[bb * bucket : bb * bucket + nrows, :],
                    in_=ow[:nrows, nfull * d_feat : (nfull + 1) * d_feat],
                )

    nc.compile()
    return nc


_CACHE = {}


def _run(embeddings, src, dst, trace=False, trace_kwargs=None):
    embeddings = np.asarray(embeddings, dtype=np.float32)
    emb2 = np.zeros((N_NODES, ROW), BF16)
    emb2[:, :D_FEAT] = embeddings.astype(BF16)
    idxtab, dst_t, rec_t, sched = _prep(
        src, dst, N_NODES, N_CORES, NODES_PER_CORE, BUCKET, N_GROUPS, WAVE
    )
    key = sched["tbg"].tobytes()
    if key not in _CACHE:
        _CACHE[key] = _build(N_NODES, D_FEAT, NODES_PER_CORE, BUCKET, N_GROUPS, sched)
    nc = _CACHE[key]

    in_maps = [
        {
            "emb": emb2,
            "idx_t": idxtab[c],
            "dst_t": dst_t[c],
            "rec_t": rec_t[c],
        }
        for c in range(N_CORES)
    ]
    res = run_bass_kernel_spmd(
        nc,
        in_maps,
        core_ids=list(range(N_CORES)),
        trace=trace,
        **(trace_kwargs or {}),
    )
    out = np.concatenate([res.results[c]["out"] for c in range(N_CORES)], axis=0)
    return out, res


def kernel(embeddings, src, dst):
    out, _ = _run(embeddings, src, dst, trace=False)
    return out


# revision 49
# speedup vs baseline: 1.1221x; 1.1221x over previous
"""GCN mean-aggregation (DGL copy_src -> mean by dst) on 8 NeuronCores.

Strategy (dst-sharded, no collectives):
  - Host: edges are assigned to the core owning their dst row (core c owns
    rows [c*12500, (c+1)*12500)).  Within a core, dst nodes form 98 buckets
    of 128; src rows are split into 4 groups of 25000 so gather indices fit
    int16 (dma_gather requirement).  Edges are sorted by
    (bucket-wave, src-group, bucket, src) and each (bucket, group) run is
    padded to a static number of 128-edge tiles (max over the 8 cores), so a
    single program serves all cores.  Pad edges gather a garbage row and are
    masked out by a zero one-hot row (dst_local = 128).
  - Embeddings are pre-cast to bf16 and padded to 256B rows ([N, 128] bf16,
    features in cols 0:64): dma_gather requires a 256B-aligned source stride.
    Only the 64 real features (128B) are gathered per edge (elem_size=64,
    elem_step=128; the stock 256B elem_size assert is relaxed at import).
  - Counts are folded host-side into a per-core reciprocal table rec[p, b] =
    1/max(indeg, 1); no count matmul on device.
  - Device (identical program per core):
      * per (wave of 8 buckets) x (src group): batched dma_gather of bf16
        embedding rows (128B each) into SBUF, rotating 4 SWDGE queues
      * per wave: ONE DVE tensor_tensor(is_equal) with stride-0 broadcast
        APs builds the bf16 one-hots for every tile in the wave
      * per edge-tile: psum[:, :64] += onehot^T @ msgs   (bf16 matmul)
      * per bucket: Activation-engine copy psum*rec -> SBUF; DMA out
  - Host: concatenate the 8 per-core [12500, 64] outputs.
"""

import sys
from contextlib import ExitStack

import numpy as np
import ml_dtypes

sys.path.insert(0, "/opt/trn_rl_repo")

import concourse.bass as bass  # noqa: E402
import concourse.mybir as mybir  # noqa: E402
import concourse.tile as tile  # noqa: E402
from concourse import bacc  # noqa: E402
from concourse.bass_utils import run_bass_kernel_spmd  # noqa: E402

# dma_gather asserts elem_size_bytes % 256 == 0, but the ISA only needs the
# source stride (elem_step) 256B-aligned; relax to 128B so we can gather just
# the 64 real bf16 features (128B) from each 256B-strided row.
import inspect as _inspect  # noqa: E402
import textwrap as _textwrap  # noqa: E402

_src = _textwrap.dedent(_inspect.getsource(bass.BassGpSimd.dma_gather))
_src = _src.replace("elem_size_bytes % 256 == 0", "elem_size_bytes % 128 == 0")
_ns = dict(bass.__dict__)
exec(_src, _ns)  # noqa: S102
bass.BassGpSimd.dma_gather = _ns["dma_gather"]

N_NODES = 100000
N_EDGES = 1000000
D_FEAT = 64
N_CORES = 8
NODES_PER_CORE = N_NODES // N_CORES  # 12500
BUCKET = 128  # dst nodes per psum bucket (= one-hot free dim)
N_GROUPS = 4  # src-row groups (int16 index range for dma_gather)
WAVE = 8  # buckets per gather wave
ROW = 128  # padded bf16 row length (256B dma_gather granularity)

BF16 = ml_dtypes.bfloat16


def _schedule(cnt_max, npc, bucket, wave):
    """Static schedule from per-(bucket, group) max edge counts.

    cnt_max: [nb, ngroups] max edge count over cores.
    Returns dict with tiles-per-region, waves, per-call and per-bucket info.
    """
    nb, ngroups = cnt_max.shape
    tbg = -(-cnt_max // 128)  # [nb, ngroups]
    for b in range(nb):
        if tbg[b].sum() == 0:
            tbg[b, 0] = 1  # ensure psum gets reset even for empty buckets

    waves = [range(w, min(w + wave, nb)) for w in range(0, nb, wave)]
    # region order: (wave, group, bucket-in-wave)
    region_tile0 = np.zeros((nb, ngroups), np.int64)
    calls = []  # [wave][group] -> (tile0, ntiles)
    t = 0
    for wv in waves:
        wcalls = []
        for g in range(ngroups):
            c0 = t
            for b in wv:
                region_tile0[b, g] = t
                t += int(tbg[b, g])
            wcalls.append((c0, t - c0))
        calls.append(wcalls)
    nt = t
    return {
        "tbg": tbg,
        "waves": waves,
        "region_tile0": region_tile0,
        "calls": calls,
        "nt": nt,
    }


def _prep(src, dst, n_nodes, n_cores, npc, bucket, ngroups, wave):
    """Sort/group/pad edges; build per-core device inputs + static schedule."""
    src = np.asarray(src, dtype=np.int64)
    dst = np.asarray(dst, dtype=np.int64)
    gsz = n_nodes // ngroups
    nb = -(-npc // bucket)
    nw = -(-nb // wave)

    core = dst // npc
    b = (dst - core * npc) // bucket
    g = src // gsz
    w = b // wave

    cnt = np.zeros((n_cores, nb, ngroups), np.int64)
    np.add.at(cnt, (core, b, g), 1)
    sched = _schedule(cnt.max(axis=0), npc, bucket, wave)
    tbg, region_tile0, nt = sched["tbg"], sched["region_tile0"], sched["nt"]
    nslot = nt * 128

    # global sort by (core, wave, group, bucket, src)
    key = (((core * nw + w) * ngroups + g) * nb + b)
    order = np.lexsort((src, key))
    ss, ks = src[order], key[order]
    dl = (dst - (core * npc + b * bucket))[order]  # dst_local in [0, bucket)

    kcnt = np.bincount(ks, minlength=n_cores * nw * ngroups * nb)
    kstart = np.zeros(kcnt.shape[0] + 1, np.int64)
    np.cumsum(kcnt, out=kstart[1:])
    rank = np.arange(ss.shape[0], dtype=np.int64) - kstart[ks]

    slot_base = region_tile0 * 128  # [nb, ngroups], within-core slot offset
    bo, go, co = b[order], g[order], core[order]
    pos = co * nslot + slot_base[bo, go] + rank

    # per-slot group id (for pad values), same for every core
    slot_group = np.zeros(nslot, np.int64)
    for bb in range(nb):
        for gg in range(ngroups):
            t0 = region_tile0[bb, gg] * 128
            slot_group[t0 : t0 + tbg[bb, gg] * 128] = gg

    src_slot = np.tile((slot_group + 1) * gsz - 1, n_cores)  # pad: last row of group
    dstloc = np.full(n_cores * nslot, float(bucket), np.float32)
    src_slot[pos] = ss
    dstloc[pos] = dl.astype(np.float32)

    idx16 = (src_slot - np.tile(slot_group * gsz, n_cores)).astype(np.int16)
    # wrapped index layout: idx j -> partition j%16, col j//16 (x8 replicas)
    idx16 = idx16.reshape(n_cores, nt * 8, 16)
    idxtab = np.ascontiguousarray(idx16.transpose(0, 2, 1))  # [C, 16, nt*8]
    idxtab = np.tile(idxtab, (1, 8, 1))  # [C, 128, nt*8]

    dst_t = np.ascontiguousarray(
        dstloc.reshape(n_cores, nt, 128).transpose(0, 2, 1)
    ).astype(BF16)  # [C, 128, nt]

    # per-core reciprocal table rec[p, b] = 1/max(indeg(core*npc + b*128 + p), 1)
    indeg = np.bincount(dst, minlength=n_nodes).astype(np.float32)
    rec = 1.0 / np.maximum(indeg, 1.0)
    nb_rows = nb * 128
    rec_t = np.ones((n_cores, nb_rows), np.float32)
    for c in range(n_cores):
        rec_t[c, :npc] = rec[c * npc : (c + 1) * npc]
    rec_t = np.ascontiguousarray(
        rec_t.reshape(n_cores, nb, 128).transpose(0, 2, 1)
    )  # [C, 128, nb]
    return idxtab, dst_t, rec_t, sched


def _build(n_nodes, d_feat, npc, bucket, ngroups, sched):
    """Build the (per-core) Bass program."""
    gsz = n_nodes // ngroups
    nb = -(-npc // bucket)
    nt = sched["nt"]
    tbg, region_tile0 = sched["tbg"], sched["region_tile0"]
    f32 = mybir.dt.float32
    bf16 = mybir.dt.bfloat16
    i16 = mybir.dt.int16

    # max tiles in any wave (for pool sizing)
    wave_spans = []
    for wvi, wv in enumerate(sched["waves"]):
        wt0 = sched["calls"][wvi][0][0]
        wt = sum(c[1] for c in sched["calls"][wvi])
        wave_spans.append((wt0, wt))

    nc = bacc.Bacc(
        "TRN2", target_bir_lowering=False, debug=False, num_swdge_queues=4
    )
    emb = nc.dram_tensor("emb", [n_nodes, ROW], bf16, kind="ExternalInput")
    idx_t = nc.dram_tensor("idx_t", [128, nt * 8], i16, kind="ExternalInput")
    dst_t = nc.dram_tensor("dst_t", [128, nt], bf16, kind="ExternalInput")
    rec_t = nc.dram_tensor("rec_t", [128, nb], f32, kind="ExternalInput")
    out = nc.dram_tensor("out", [npc, d_feat], f32, kind="ExternalOutput")

    with tile.TileContext(nc) as tc, ExitStack() as ctx:
        const_p = ctx.enter_context(tc.tile_pool(name="const", bufs=1))
        idx_p = ctx.enter_context(tc.tile_pool(name="idx", bufs=1))
        msgs_p = ctx.enter_context(tc.tile_pool(name="msgs", bufs=3))
        oh_p = ctx.enter_context(tc.tile_pool(name="oh", bufs=3))
        ps_p = ctx.enter_context(tc.tile_pool(name="ps", bufs=8, space="PSUM"))
        outp_p = ctx.enter_context(tc.tile_pool(name="outp", bufs=4))

        # idx/dst tables are loaded per wave so wave 0's gathers start early
        idxall = idx_p.tile([128, nt * 8], i16)
        dstall = idx_p.tile([128, nt], bf16)
        # dummy 128-idx gather warms up the SWDGE ucode library while the
        # idx tables stream in (group 0 so even garbage idx stay in bounds)
        warm = const_p.tile([128, 16], i16)
        nc.gpsimd.memset(warm[:], 0)
        scratch = const_p.tile([128, d_feat], bf16)
        nc.gpsimd.dma_gather(
            out_ap=scratch[:].rearrange("p (t e) -> p t e", e=d_feat),
            in_ap=emb[0:gsz, 0:d_feat],
            idxs_ap=warm[:, 0:8],
            num_idxs=128,
            num_idxs_reg=128,
            elem_size=d_feat,
            elem_step=ROW,
            queue_num=0,
        )
        # two-stage load: wave 0's slice first (its gathers start right away),
        # then the rest in one DMA each (no per-wave HWDGE contention with
        # gather drain at wave boundaries)
        w0t = sum(c[1] for c in sched["calls"][0])
        nc.sync.dma_start(out=idxall[:, : w0t * 8], in_=idx_t[:, : w0t * 8])
        nc.sync.dma_start(out=dstall[:, :w0t], in_=dst_t[:, :w0t])
        nc.sync.dma_start(out=idxall[:, w0t * 8 :], in_=idx_t[:, w0t * 8 :])
        nc.sync.dma_start(out=dstall[:, w0t:], in_=dst_t[:, w0t:])
        recall = idx_p.tile([128, nb], f32)
        nc.sync.dma_start(out=recall[:], in_=rec_t[:, :])
        iota_i = const_p.tile([128, bucket], mybir.dt.int32)
        nc.gpsimd.iota(iota_i[:], pattern=[[1, bucket]], base=0, channel_multiplier=0)
        iota_b = const_p.tile([128, bucket], bf16)
        nc.vector.tensor_copy(out=iota_b[:], in_=iota_i[:])

        qn = 0
        for wvi, wv in enumerate(sched["waves"]):
            wt0, wt = wave_spans[wvi]
            # one batched one-hot build for the whole wave:
            # oh[p, t, f] = (iota[p, f] == dstall[p, wt0 + t])
            oh = oh_p.tile([128, wt * bucket], bf16, tag="oh")
            nc.vector.tensor_tensor(
                out=oh[:].rearrange("p (t f) -> p t f", f=bucket),
                in0=iota_b[:].unsqueeze(1).broadcast_to([128, wt, bucket]),
                in1=dstall[:, wt0 : wt0 + wt]
                .unsqueeze(2)
                .broadcast_to([128, wt, bucket]),
                op=mybir.AluOpType.is_equal,
            )
            msgs = {}
            call0 = {}
            for gg in range(ngroups):
                t0, ntl = sched["calls"][wvi][gg]
                call0[gg] = t0
                if ntl == 0:
                    continue
                m = msgs_p.tile([128, ntl * d_feat], bf16, tag=f"msgs{gg}")
                msgs[gg] = m
                # dma_gather is limited to 1024 indices (8 tiles) per call
                for sc in range(0, ntl, 8):
                    k = min(8, ntl - sc)
                    ts = t0 + sc
                    nc.gpsimd.dma_gather(
                        out_ap=m[
                            :, sc * d_feat : (sc + k) * d_feat
                        ].rearrange("p (t e) -> p t e", e=d_feat),
                        in_ap=emb[gg * gsz : (gg + 1) * gsz, 0:d_feat],
                        idxs_ap=idxall[:, ts * 8 : (ts + k) * 8],
                        num_idxs=k * 128,
                        num_idxs_reg=k * 128,
                        elem_size=d_feat,
                        elem_step=ROW,
                        queue_num=qn,
                    )
                    qn = (qn + 1) % 4
            # group-major matmul order: buckets' psum chains interleave, so
            # g0..g2 matmuls run while later groups' gathers are in flight
            # (bucket-major would serialize the whole wave behind g3's DMA)
            psums = {}
            for bb in wv:
                psum = ps_p.tile([bucket, d_feat], f32)
                psums[bb] = psum
            first = {bb: True for bb in wv}
            last_pass = {
                bb: max(
                    (gg, int(tbg[bb, gg]) - 1)
                    for gg in range(ngroups)
                    if int(tbg[bb, gg]) > 0
                )
                for bb in wv
            }
            for gg in range(ngroups):
                for bb in wv:
                    for j in range(int(tbg[bb, gg])):
                        t = int(region_tile0[bb, gg]) + j
                        off = int(t - call0[gg]) * d_feat
                        nc.tensor.matmul(
                            out=psums[bb][:],
                            lhsT=oh[:, (t - wt0) * bucket : (t - wt0 + 1) * bucket],
                            rhs=msgs[gg][:, off : off + d_feat],
                            start=first[bb],
                            stop=(gg, j) == last_pass[bb],
                        )
                        first[bb] = False
            # one output tile per wave; a single strided DMA writes all rows
            nw_b = len(wv)
            ow = outp_p.tile([bucket, nw_b * d_feat], f32, tag="ow")
            for bi, bb in enumerate(wv):
                nc.scalar.mul(
                    out=ow[:, bi * d_feat : (bi + 1) * d_feat],
                    in_=psums[bb][:],
                    mul=recall[:, bb : bb + 1],
                )
            r0 = wv[0] * bucket
            nfull = min(nw_b, (npc - r0) // bucket)
            if nfull > 0:
                nc.sync.dma_start(
                    out=out[r0 : r0 + nfull * bucket, :].rearrange(
                        "(b p) f -> p b f", b=nfull
                    ),
                    in_=ow[:, : nfull * d_feat].rearrange(
                        "p (b f) -> p b f", f=d_feat
                    ),
                )
            if nfull < nw_b:  # partial last bucket
                bb = wv[nfull]
                nrows = npc - bb * bucket
                nc.sync.dma_start(
                    out=out[bb * bucket : bb * bucket + nrows, :],
                    in_=ow[:nrows, nfull * d_feat : (nfull + 1) * d_feat],
                )

    nc.compile()
    return nc


_CACHE = {}


def _run(embeddings, src, dst, trace=False, trace_kwargs=None):
    embeddings = np.asarray(embeddings, dtype=np.float32)
    emb2 = np.zeros((N_NODES, ROW), BF16)
    emb2[:, :D_FEAT] = embeddings.astype(BF16)
    idxtab, dst_t, rec_t, sched = _prep(
        src, dst, N_NODES, N_CORES, NODES_PER_CORE, BUCKET, N_GROUPS, WAVE
    )
    key = sched["tbg"].tobytes()
    if key not in _CACHE:
        _CACHE[key] = _build(N_NODES, D_FEAT, NODES_PER_CORE, BUCKET, N_GROUPS, sched)
    nc = _CACHE[key]

    in_maps = [
        {
            "emb": emb2,
            "idx_t": idxtab[c],
            "dst_t": dst_t[c],
            "rec_t": rec_t[c],
        }
        for c in range(N_CORES)
    ]
    res = run_bass_kernel_spmd(
        nc,
        in_maps,
        core_ids=list(range(N_CORES)),
        trace=trace,
        **(trace_kwargs or {}),
    )
    out = np.concatenate([res.results[c]["out"] for c in range(N_CORES)], axis=0)
    return out, res


def kernel(embeddings, src, dst):
    out, _ = _run(embeddings, src, dst, trace=False)
    return out


# revision 50
# speedup vs baseline: 1.1337x; 1.0103x over previous
"""GCN mean-aggregation (DGL copy_src -> mean by dst) on 8 NeuronCores.

Strategy (dst-sharded, no collectives):
  - Host: edges are assigned to the core owning their dst row (core c owns
    rows [c*12500, (c+1)*12500)).  Within a core, dst nodes form 98 buckets
    of 128; src rows are split into 4 groups of 25000 so gather indices fit
    int16 (dma_gather requirement).  Edges are sorted by
    (bucket-wave, src-group, bucket, src) and each (bucket, group) run is
    padded to a static number of 128-edge tiles (max over the 8 cores), so a
    single program serves all cores.  Pad edges gather a garbage row and are
    masked out by a zero one-hot row (dst_local = 128).
  - Embeddings are pre-cast to bf16 and padded to 256B rows ([N, 128] bf16,
    features in cols 0:64): dma_gather requires a 256B-aligned source stride.
    Only the 64 real features (128B) are gathered per edge (elem_size=64,
    elem_step=128; the stock 256B elem_size assert is relaxed at import).
  - Counts are folded host-side into a per-core reciprocal table rec[p, b] =
    1/max(indeg, 1); no count matmul on device.
  - Device (identical program per core):
      * per (wave of 8 buckets) x (src group): batched dma_gather of bf16
        embedding rows (128B each) into SBUF, rotating 4 SWDGE queues
      * per wave: ONE DVE tensor_tensor(is_equal) with stride-0 broadcast
        APs builds the bf16 one-hots for every tile in the wave
      * per edge-tile: psum[:, :64] += onehot^T @ msgs   (bf16 matmul)
      * per bucket: Activation-engine copy psum*rec -> SBUF; DMA out
  - Host: concatenate the 8 per-core [12500, 64] outputs.
"""

import sys
from contextlib import ExitStack

import numpy as np
import ml_dtypes

sys.path.insert(0, "/opt/trn_rl_repo")

import concourse.bass as bass  # noqa: E402
import concourse.mybir as mybir  # noqa: E402
import concourse.tile as tile  # noqa: E402
from concourse import bacc  # noqa: E402
from concourse.bass_utils import run_bass_kernel_spmd  # noqa: E402

# dma_gather asserts elem_size_bytes % 256 == 0, but the ISA only needs the
# source stride (elem_step) 256B-aligned; relax to 128B so we can gather just
# the 64 real bf16 features (128B) from each 256B-strided row.
import inspect as _inspect  # noqa: E402
import textwrap as _textwrap  # noqa: E402

_src = _textwrap.dedent(_inspect.getsource(bass.BassGpSimd.dma_gather))
_src = _src.replace("elem_size_bytes % 256 == 0", "elem_size_bytes % 128 == 0")
_ns = dict(bass.__dict__)
exec(_src, _ns)  # noqa: S102
bass.BassGpSimd.dma_gather = _ns["dma_gather"]

N_NODES = 100000
N_EDGES = 1000000
D_FEAT = 64
N_CORES = 8
NODES_PER_CORE = N_NODES // N_CORES  # 12500
BUCKET = 128  # dst nodes per psum bucket (= one-hot free dim)
N_GROUPS = 4  # src-row groups (int16 index range for dma_gather)
WAVE = 8  # buckets per gather wave
ROW = 128  # padded bf16 row length (256B dma_gather granularity)

BF16 = ml_dtypes.bfloat16


def _schedule(cnt_max, npc, bucket, wave):
    """Static schedule from per-(bucket, group) max edge counts.

    cnt_max: [nb, ngroups] max edge count over cores.
    Returns dict with tiles-per-region, waves, per-call and per-bucket info.
    """
    nb, ngroups = cnt_max.shape
    tbg = -(-cnt_max // 128)  # [nb, ngroups]
    for b in range(nb):
        if tbg[b].sum() == 0:
            tbg[b, 0] = 1  # ensure psum gets reset even for empty buckets

    waves = [range(w, min(w + wave, nb)) for w in range(0, nb, wave)]
    # region order: (wave, group, bucket-in-wave)
    region_tile0 = np.zeros((nb, ngroups), np.int64)
    calls = []  # [wave][group] -> (tile0, ntiles)
    t = 0
    for wv in waves:
        wcalls = []
        for g in range(ngroups):
            c0 = t
            for b in wv:
                region_tile0[b, g] = t
                t += int(tbg[b, g])
            wcalls.append((c0, t - c0))
        calls.append(wcalls)
    nt = t
    return {
        "tbg": tbg,
        "waves": waves,
        "region_tile0": region_tile0,
        "calls": calls,
        "nt": nt,
    }


def _prep(src, dst, n_nodes, n_cores, npc, bucket, ngroups, wave):
    """Sort/group/pad edges; build per-core device inputs + static schedule."""
    src = np.asarray(src, dtype=np.int64)
    dst = np.asarray(dst, dtype=np.int64)
    gsz = n_nodes // ngroups
    nb = -(-npc // bucket)
    nw = -(-nb // wave)

    core = dst // npc
    b = (dst - core * npc) // bucket
    g = src // gsz
    w = b // wave

    cnt = np.zeros((n_cores, nb, ngroups), np.int64)
    np.add.at(cnt, (core, b, g), 1)
    sched = _schedule(cnt.max(axis=0), npc, bucket, wave)
    tbg, region_tile0, nt = sched["tbg"], sched["region_tile0"], sched["nt"]
    nslot = nt * 128

    # global sort by (core, wave, group, bucket, src)
    key = (((core * nw + w) * ngroups + g) * nb + b)
    order = np.lexsort((src, key))
    ss, ks = src[order], key[order]
    dl = (dst - (core * npc + b * bucket))[order]  # dst_local in [0, bucket)

    kcnt = np.bincount(ks, minlength=n_cores * nw * ngroups * nb)
    kstart = np.zeros(kcnt.shape[0] + 1, np.int64)
    np.cumsum(kcnt, out=kstart[1:])
    rank = np.arange(ss.shape[0], dtype=np.int64) - kstart[ks]

    slot_base = region_tile0 * 128  # [nb, ngroups], within-core slot offset
    bo, go, co = b[order], g[order], core[order]
    pos = co * nslot + slot_base[bo, go] + rank

    # per-slot group id (for pad values), same for every core
    slot_group = np.zeros(nslot, np.int64)
    for bb in range(nb):
        for gg in range(ngroups):
            t0 = region_tile0[bb, gg] * 128
            slot_group[t0 : t0 + tbg[bb, gg] * 128] = gg

    src_slot = np.tile((slot_group + 1) * gsz - 1, n_cores)  # pad: last row of group
    dstloc = np.full(n_cores * nslot, float(bucket), np.float32)
    src_slot[pos] = ss
    dstloc[pos] = dl.astype(np.float32)

    idx16 = (src_slot - np.tile(slot_group * gsz, n_cores)).astype(np.int16)
    # wrapped index layout: idx j -> partition j%16, col j//16 (x8 replicas)
    idx16 = idx16.reshape(n_cores, nt * 8, 16)
    idxtab = np.ascontiguousarray(idx16.transpose(0, 2, 1))  # [C, 16, nt*8]
    idxtab = np.tile(idxtab, (1, 8, 1))  # [C, 128, nt*8]

    dst_t = np.ascontiguousarray(
        dstloc.reshape(n_cores, nt, 128).transpose(0, 2, 1)
    ).astype(BF16)  # [C, 128, nt]

    # per-core reciprocal table rec[p, b] = 1/max(indeg(core*npc + b*128 + p), 1)
    indeg = np.bincount(dst, minlength=n_nodes).astype(np.float32)
    rec = 1.0 / np.maximum(indeg, 1.0)
    nb_rows = nb * 128
    rec_t = np.ones((n_cores, nb_rows), np.float32)
    for c in range(n_cores):
        rec_t[c, :npc] = rec[c * npc : (c + 1) * npc]
    rec_t = np.ascontiguousarray(
        rec_t.reshape(n_cores, nb, 128).transpose(0, 2, 1)
    )  # [C, 128, nb]
    return idxtab, dst_t, rec_t, sched


def _build(n_nodes, d_feat, npc, bucket, ngroups, sched):
    """Build the (per-core) Bass program."""
    gsz = n_nodes // ngroups
    nb = -(-npc // bucket)
    nt = sched["nt"]
    tbg, region_tile0 = sched["tbg"], sched["region_tile0"]
    f32 = mybir.dt.float32
    bf16 = mybir.dt.bfloat16
    i16 = mybir.dt.int16

    # max tiles in any wave (for pool sizing)
    wave_spans = []
    for wvi, wv in enumerate(sched["waves"]):
        wt0 = sched["calls"][wvi][0][0]
        wt = sum(c[1] for c in sched["calls"][wvi])
        wave_spans.append((wt0, wt))

    nc = bacc.Bacc(
        "TRN2", target_bir_lowering=False, debug=False, num_swdge_queues=4
    )
    emb = nc.dram_tensor("emb", [n_nodes, ROW], bf16, kind="ExternalInput")
    idx_t = nc.dram_tensor("idx_t", [128, nt * 8], i16, kind="ExternalInput")
    dst_t = nc.dram_tensor("dst_t", [128, nt], bf16, kind="ExternalInput")
    rec_t = nc.dram_tensor("rec_t", [128, nb], f32, kind="ExternalInput")
    out = nc.dram_tensor("out", [npc, d_feat], f32, kind="ExternalOutput")

    with tile.TileContext(nc) as tc, ExitStack() as ctx:
        const_p = ctx.enter_context(tc.tile_pool(name="const", bufs=1))
        idx_p = ctx.enter_context(tc.tile_pool(name="idx", bufs=1))
        msgs_p = ctx.enter_context(tc.tile_pool(name="msgs", bufs=3))
        oh_p = ctx.enter_context(tc.tile_pool(name="oh", bufs=3))
        ps_p = ctx.enter_context(tc.tile_pool(name="ps", bufs=8, space="PSUM"))
        outp_p = ctx.enter_context(tc.tile_pool(name="outp", bufs=4))

        # idx/dst tables are loaded per wave so wave 0's gathers start early
        idxall = idx_p.tile([128, nt * 8], i16)
        dstall = idx_p.tile([128, nt], bf16)
        # dummy 128-idx gather warms up the SWDGE ucode library while the
        # idx tables stream in (group 0 so even garbage idx stay in bounds)
        warm = const_p.tile([128, 16], i16)
        nc.vector.memset(warm[:], 0)
        scratch = const_p.tile([128, d_feat], bf16)
        nc.gpsimd.dma_gather(
            out_ap=scratch[:].rearrange("p (t e) -> p t e", e=d_feat),
            in_ap=emb[0:gsz, 0:d_feat],
            idxs_ap=warm[:, 0:8],
            num_idxs=128,
            num_idxs_reg=128,
            elem_size=d_feat,
            elem_step=ROW,
            queue_num=0,
        )
        # two-stage load: wave 0's slice first (its gathers start right away),
        # then the rest in one DMA each (no per-wave HWDGE contention with
        # gather drain at wave boundaries)
        w0t = sum(c[1] for c in sched["calls"][0])
        nc.sync.dma_start(out=idxall[:, : w0t * 8], in_=idx_t[:, : w0t * 8])
        nc.sync.dma_start(out=dstall[:, :w0t], in_=dst_t[:, :w0t])
        nc.sync.dma_start(out=idxall[:, w0t * 8 :], in_=idx_t[:, w0t * 8 :])
        nc.sync.dma_start(out=dstall[:, w0t:], in_=dst_t[:, w0t:])
        recall = idx_p.tile([128, nb], f32)
        nc.sync.dma_start(out=recall[:], in_=rec_t[:, :])
        iota_i = const_p.tile([128, bucket], mybir.dt.int32)
        nc.gpsimd.iota(iota_i[:], pattern=[[1, bucket]], base=0, channel_multiplier=0)
        iota_b = const_p.tile([128, bucket], bf16)
        nc.vector.tensor_copy(out=iota_b[:], in_=iota_i[:])

        qn = 0
        for wvi, wv in enumerate(sched["waves"]):
            wt0, wt = wave_spans[wvi]
            # one batched one-hot build for the whole wave:
            # oh[p, t, f] = (iota[p, f] == dstall[p, wt0 + t])
            oh = oh_p.tile([128, wt * bucket], bf16, tag="oh")
            nc.vector.tensor_tensor(
                out=oh[:].rearrange("p (t f) -> p t f", f=bucket),
                in0=iota_b[:].unsqueeze(1).broadcast_to([128, wt, bucket]),
                in1=dstall[:, wt0 : wt0 + wt]
                .unsqueeze(2)
                .broadcast_to([128, wt, bucket]),
                op=mybir.AluOpType.is_equal,
            )
            msgs = {}
            call0 = {}
            for gg in range(ngroups):
                t0, ntl = sched["calls"][wvi][gg]
                call0[gg] = t0
                if ntl == 0:
                    continue
                m = msgs_p.tile([128, ntl * d_feat], bf16, tag=f"msgs{gg}")
                msgs[gg] = m
                # dma_gather is limited to 1024 indices (8 tiles) per call
                for sc in range(0, ntl, 8):
                    k = min(8, ntl - sc)
                    ts = t0 + sc
                    nc.gpsimd.dma_gather(
                        out_ap=m[
                            :, sc * d_feat : (sc + k) * d_feat
                        ].rearrange("p (t e) -> p t e", e=d_feat),
                        in_ap=emb[gg * gsz : (gg + 1) * gsz, 0:d_feat],
                        idxs_ap=idxall[:, ts * 8 : (ts + k) * 8],
                        num_idxs=k * 128,
                        num_idxs_reg=k * 128,
                        elem_size=d_feat,
                        elem_step=ROW,
                        queue_num=qn,
                    )
                    qn = (qn + 1) % 4
            # group-major matmul order: buckets' psum chains interleave, so
            # g0..g2 matmuls run while later groups' gathers are in flight
            # (bucket-major would serialize the whole wave behind g3's DMA)
            psums = {}
            for bb in wv:
                psum = ps_p.tile([bucket, d_feat], f32)
                psums[bb] = psum
            first = {bb: True for bb in wv}
            last_pass = {
                bb: max(
                    (gg, int(tbg[bb, gg]) - 1)
                    for gg in range(ngroups)
                    if int(tbg[bb, gg]) > 0
                )
                for bb in wv
            }
            for gg in range(ngroups):
                for bb in wv:
                    for j in range(int(tbg[bb, gg])):
                        t = int(region_tile0[bb, gg]) + j
                        off = int(t - call0[gg]) * d_feat
                        nc.tensor.matmul(
                            out=psums[bb][:],
                            lhsT=oh[:, (t - wt0) * bucket : (t - wt0 + 1) * bucket],
                            rhs=msgs[gg][:, off : off + d_feat],
                            start=first[bb],
                            stop=(gg, j) == last_pass[bb],
                        )
                        first[bb] = False
            # one output tile per wave; a single strided DMA writes all rows
            nw_b = len(wv)
            ow = outp_p.tile([bucket, nw_b * d_feat], f32, tag="ow")
            for bi, bb in enumerate(wv):
                nc.scalar.mul(
                    out=ow[:, bi * d_feat : (bi + 1) * d_feat],
                    in_=psums[bb][:],
                    mul=recall[:, bb : bb + 1],
                )
            r0 = wv[0] * bucket
            nfull = min(nw_b, (npc - r0) // bucket)
            if nfull > 0:
                nc.sync.dma_start(
                    out=out[r0 : r0 + nfull * bucket, :].rearrange(
                        "(b p) f -> p b f", b=nfull
                    ),
                    in_=ow[:, : nfull * d_feat].rearrange(
                        "p (b f) -> p b f", f=d_feat
                    ),
                )
            if nfull < nw_b:  # partial last bucket
                bb = wv[nfull]
                nrows = npc - bb * bucket
                nc.sync.dma_start(
                    out=out[bb * bucket : bb * bucket + nrows, :],
                    in_=ow[:nrows, nfull * d_feat : (nfull + 1) * d_feat],
                )

    nc.compile()
    return nc


_CACHE = {}


def _run(embeddings, src, dst, trace=False, trace_kwargs=None):
    embeddings = np.asarray(embeddings, dtype=np.float32)
    emb2 = np.zeros((N_NODES, ROW), BF16)
    emb2[:, :D_FEAT] = embeddings.astype(BF16)
    idxtab, dst_t, rec_t, sched = _prep(
        src, dst, N_NODES, N_CORES, NODES_PER_CORE, BUCKET, N_GROUPS, WAVE
    )
    key = sched["tbg"].tobytes()
    if key not in _CACHE:
        _CACHE[key] = _build(N_NODES, D_FEAT, NODES_PER_CORE, BUCKET, N_GROUPS, sched)
    nc = _CACHE[key]

    in_maps = [
        {
            "emb": emb2,
            "idx_t": idxtab[c],
            "dst_t": dst_t[c],
            "rec_t": rec_t[c],
        }
        for c in range(N_CORES)
    ]
    res = run_bass_kernel_spmd(
        nc,
        in_maps,
        core_ids=list(range(N_CORES)),
        trace=trace,
        **(trace_kwargs or {}),
    )
    out = np.concatenate([res.results[c]["out"] for c in range(N_CORES)], axis=0)
    return out, res


def kernel(embeddings, src, dst):
    out, _ = _run(embeddings, src, dst, trace=False)
    return out
